# revision 6
# baseline (speedup 1.0000x reference)
"""Trainium2 Bass kernel for the sparse-attention scoring module (v5).

The reference collapses algebraically: with w = W_attn.T @ v split into
w1 (decoder half) / w2 (encoder half) and c1 = av @ w1 + b_attn . v,
    score[b,t] = enc[t,b,:] . w2 + c1[b]   -> /weight -> mask -> softmax.

The device is a pure row-dot machine; everything else rides the host:

  1. enc ships bf16 (2e-2 rel-err budget >> ~1e-3 bf16 rounding).
  2. Masked (b,t) positions (output exactly 0) are never shipped; the
     host compacts unmasked rows only.
  3. Batches are assigned to cores by greedy row balancing (the device
     has no batch structure), so every core ships the same minimal
     256-multiple row count (compile cached per NT).
  4. The 1/weight scale is folded into the shipped data host-side.
  5. The softmax (exp + per-batch normalize, ~0.5% of the FLOPs) runs on
     the host on the 16 KB/core result; the device tail per half is one
     DVE add + the out DMA.

Layout: shards are e-major [E2, NT*128] bf16 so the w2 dot contracts
over SBUF partitions on the tensor engine. Each 128-wide e-chunk ships
as TWO half-row DMAs on opposite HWDGE rings (16 balanced transfers);
each half is its own PSUM-bank accumulation group (start on the first
of its NT/2 matmuls, stop on the last) so no zero region is ever
restarted before it is read and the PE latches onto each half as it
lands. ACT/DVE fold partials into SBUF between arrivals. w2 loads first
on the sync HWDGE ring so the PE never waits on the slow SWDGE path.

Row mapping: flat row i = j*128 + p is the i-th entry of the core's
concatenated unmasked (b, t) list; the host adds c1[b]/weight,
exponentiates, segment-sums per batch, normalizes, and scatters into
the zeroed [B, T] output.
"""

import numpy as np

N_CORES = 8
B, T, E2, D, A = 64, 1024, 1024, 1024, 1024
NE = E2 // 128                # 8 e-chunks of 128

_CACHE = {}


def _build_nc(nt):
    import concourse.bass as bass
    import concourse.tile as tile
    from concourse import bacc, mybir
    from contextlib import ExitStack

    rows = nt * 128
    half = rows // 2
    nh = nt // 2
    f32 = mybir.dt.float32
    bf16 = mybir.dt.bfloat16
    nc = bacc.Bacc("TRN2", target_bir_lowering=False, debug=False,
                   num_devices=N_CORES)

    encT = nc.dram_tensor("encT", [E2, rows], bf16, kind="ExternalInput").ap()
    w2sb = nc.dram_tensor("w2sb", [128, 2 * NE], bf16, kind="ExternalInput").ap()
    out = nc.dram_tensor("out", [128, nt], f32, kind="ExternalOutput").ap()

    with tile.TileContext(nc) as tc, ExitStack() as ctx:
        const = ctx.enter_context(tc.tile_pool(name="const", bufs=1))
        encp = ctx.enter_context(tc.tile_pool(name="encp", bufs=6))
        accp = ctx.enter_context(tc.tile_pool(name="accp", bufs=2))
        psump = ctx.enter_context(tc.tile_pool(name="psump", bufs=6, space="PSUM"))

        # w2 is tiny and gates every matmul: load it on the fast sync
        # HWDGE ring ahead of that ring's first enc transfer.
        w2t = const.tile([128, 2 * NE], bf16)
        nc.sync.dma_start(w2t[:], w2sb)
        fin = const.tile([128, nt], f32)

        acc = [None, None]
        for ei in range(NE):
            et = encp.tile([128, rows], bf16, tag="enct")
            base = ei * 128 * rows
            for h in range(2):
                src = bass.AP(encT.tensor, base + h * half,
                              [[rows, 128], [1, half]])
                eng = nc.scalar if h == 0 else nc.sync
                eng.dma_start(et[:, h * half:(h + 1) * half], src)
                pp = psump.tile([128, nh], f32, tag="pp")
                for jj in range(nh):
                    j = h * nh + jj
                    nc.tensor.matmul(
                        pp[:, jj:jj + 1],
                        lhsT=et[:, j * 128:(j + 1) * 128],
                        rhs=w2t[:, 2 * ei:2 * ei + 1],
                        start=(jj == 0), stop=(jj == nh - 1),
                    )
                if ei == 0:
                    nacc = accp.tile([128, nh], f32, tag=f"acc{h}")
                    nc.scalar.copy(nacc[:], pp[:])
                    acc[h] = nacc
                elif ei == NE - 1:
                    # final add lands in the shared output tile so each
                    # half's out DMA fires as soon as its add is done
                    nc.vector.tensor_add(fin[:, h * nh:(h + 1) * nh],
                                         acc[h][:], pp[:])
                    eng2 = nc.sync if h == 0 else nc.scalar
                    eng2.dma_start(
                        bass.AP(out.tensor, h * nh, [[nt, 128], [1, nh]]),
                        fin[:, h * nh:(h + 1) * nh])
                else:
                    nacc = accp.tile([128, nh], f32, tag=f"acc{h}")
                    nc.vector.tensor_add(nacc[:], acc[h][:], pp[:])
                    acc[h] = nacc

    nc.compile()
    return nc


def _get_nc(nt):
    if nt not in _CACHE:
        _CACHE[nt] = _build_nc(nt)
    return _CACHE[nt]


def _distance_weight(time_step: int, max_len: int) -> np.ndarray:
    left = np.arange(time_step, 0, -1) + 2
    right = np.arange(max_len - time_step) + 2
    return np.log2(np.concatenate([left, right]).astype(np.float32))


def host_prep(attention_vector, encoder_outputs, W_attn, b_attn, v, mask,
              time_step, max_len):
    """Host-side prep: algebraic collapse, 1/weight fold, balanced
    mask compaction across cores."""
    import ml_dtypes

    av = np.ascontiguousarray(np.asarray(attention_vector, dtype=np.float32))
    enc = np.asarray(encoder_outputs, dtype=np.float32)
    W = np.asarray(W_attn, dtype=np.float32)
    bb = np.asarray(b_attn, dtype=np.float32)
    vv = np.asarray(v, dtype=np.float32)
    mk = np.asarray(mask) != 0
    ts = int(time_step)
    ml = int(max_len)
    assert av.shape == (B, D) and enc.shape == (T, B, E2)
    assert W.shape == (A, D + E2) and mk.shape == (B, T) and ml == T

    w = W.T @ vv                                   # [D+E2]
    w1, w2 = w[:D], np.ascontiguousarray(w[D:])
    bv = np.float32(bb @ vv)
    c1 = (av @ w1 + bv).astype(np.float32)         # [B]
    weight = _distance_weight(ts, ml)              # [T]
    winv = (np.float32(1.0) / weight).astype(np.float32)

    # Greedy batch->core assignment balancing total unmasked rows.
    counts = mk.sum(axis=1)                        # [B]
    order = np.argsort(-counts, kind="stable")
    bins = [[] for _ in range(N_CORES)]
    tot = np.zeros(N_CORES, dtype=np.int64)
    for b in order:
        i = int(tot.argmin())
        bins[i].append(int(b))
        tot[i] += counts[b]
    rows = int(-(-tot.max() // 256)) * 256
    rows = min(max(rows, 256), B * T // N_CORES)
    nt = rows // 128

    g_of, t_of, rep, seg = [], [], [], []
    for c in range(N_CORES):
        gs, tls, rp, off = [], [], [], [0]
        for i, b in enumerate(bins[c]):
            tl = np.nonzero(mk[b])[0]
            gs.append(np.full(len(tl), b, np.int64))
            tls.append(tl)
            rp.append(np.full(len(tl), i, np.int64))
            off.append(off[-1] + len(tl))
        pad = rows - off[-1]
        gs.append(np.full(pad, bins[c][0], np.int64))
        tls.append(np.zeros(pad, np.int64))
        g_of.append(np.concatenate(gs))
        t_of.append(np.concatenate(tls))
        rep.append(np.concatenate(rp))
        seg.append(np.asarray(off))

    # w2 chunks at even bf16 columns (keeps every moving-operand slice
    # 4-byte aligned): w2sb[p, 2*ei] = w2[ei*128 + p]
    w2sb = np.zeros((128, 2 * NE), dtype=ml_dtypes.bfloat16)
    w2sb[:, 0::2] = w2.reshape(NE, 128).T.astype(ml_dtypes.bfloat16)

    # e-major transpose with the 1/weight scale folded in, then bf16.
    encT_all = (enc.transpose(2, 1, 0) * winv[None, None, :]).astype(
        ml_dtypes.bfloat16)                        # [E2, B, T]

    in_maps = []
    for c in range(N_CORES):
        shard = np.ascontiguousarray(encT_all[:, g_of[c], t_of[c]])
        in_maps.append({"encT": shard, "w2sb": w2sb})
    meta = dict(nt=nt, rows=rows, g_of=g_of, t_of=t_of, rep=rep, seg=seg,
                c1=c1, winv=winv)
    return in_maps, meta


def host_post(raws, meta):
    """raw[p, j] = dot/weight for flat row i = j*128 + p. Add c1[b]/weight,
    exp, segment-sum per batch, normalize, scatter to [B, T]."""
    rows = meta["rows"]
    c1, winv = meta["c1"], meta["winv"]
    attn = np.zeros((B, T), dtype=np.float32)
    for c, raw in enumerate(raws):
        seg = meta["seg"][c]
        n = int(seg[-1])
        g = meta["g_of"][c][:n]
        t = meta["t_of"][c][:n]
        flat = np.asarray(raw, np.float32).T.reshape(rows)[:n]
        e = np.exp(flat + c1[g] * winv[t]).astype(np.float32)
        tot = np.add.reduceat(e.astype(np.float64), seg[:-1])
        vals = (e / tot[meta["rep"][c]]).astype(np.float32)
        attn[g, t] = vals
    return attn


def kernel(attention_vector, encoder_outputs, W_attn, b_attn, v, mask,
           time_step, max_len) -> np.ndarray:
    from concourse.bass_utils import run_bass_kernel_spmd

    in_maps, meta = host_prep(attention_vector, encoder_outputs, W_attn,
                              b_attn, v, mask, time_step, max_len)
    nc = _get_nc(meta["nt"])
    res = run_bass_kernel_spmd(nc, in_maps, list(range(N_CORES)))
    raws = [res.results[c]["out"] for c in range(N_CORES)]
    attn = host_post(raws, meta)
    return attn[:, None, :].astype(np.float32)


# revision 7
# speedup vs baseline: 1.0085x; 1.0085x over previous
"""Trainium2 Bass kernel for the sparse-attention scoring module (v5).

The reference collapses algebraically: with w = W_attn.T @ v split into
w1 (decoder half) / w2 (encoder half) and c1 = av @ w1 + b_attn . v,
    score[b,t] = enc[t,b,:] . w2 + c1[b]   -> /weight -> mask -> softmax.

The device is a pure row-dot machine; everything else rides the host:

  1. enc ships bf16 (2e-2 rel-err budget >> ~1e-3 bf16 rounding).
  2. Masked (b,t) positions (output exactly 0) are never shipped; the
     host compacts unmasked rows only.
  3. Batches are assigned to cores by greedy row balancing (the device
     has no batch structure), so every core ships the same minimal
     256-multiple row count (compile cached per NT).
  4. The 1/weight scale is folded into the shipped data host-side.
  5. The softmax (exp + per-batch normalize, ~0.5% of the FLOPs) runs on
     the host on the 16 KB/core result; the device tail per half is one
     DVE add + the out DMA.

Layout: shards are e-major [E2, NT*128] bf16 so the w2 dot contracts
over SBUF partitions on the tensor engine. Each 128-wide e-chunk ships
as TWO half-row DMAs on opposite HWDGE rings (16 balanced transfers);
each half is its own PSUM-bank accumulation group (start on the first
of its NT/2 matmuls, stop on the last) so no zero region is ever
restarted before it is read and the PE latches onto each half as it
lands. ACT/DVE fold partials into SBUF between arrivals. w2 loads first
on the sync HWDGE ring so the PE never waits on the slow SWDGE path.

Row mapping: flat row i = j*128 + p is the i-th entry of the core's
concatenated unmasked (b, t) list; the host adds c1[b]/weight,
exponentiates, segment-sums per batch, normalizes, and scatters into
the zeroed [B, T] output.
"""

import numpy as np

N_CORES = 8
B, T, E2, D, A = 64, 1024, 1024, 1024, 1024
NE = E2 // 128                # 8 e-chunks of 128

_CACHE = {}


def _build_nc(nt):
    import concourse.bass as bass
    import concourse.tile as tile
    from concourse import bacc, mybir
    from contextlib import ExitStack

    rows = nt * 128
    half = rows // 2
    nh = nt // 2
    f32 = mybir.dt.float32
    bf16 = mybir.dt.bfloat16
    nc = bacc.Bacc("TRN2", target_bir_lowering=False, debug=False,
                   num_devices=N_CORES)

    encT = nc.dram_tensor("encT", [E2, rows], bf16, kind="ExternalInput").ap()
    w2sb = nc.dram_tensor("w2sb", [128, 2 * NE], bf16, kind="ExternalInput").ap()
    out = nc.dram_tensor("out", [128, nt], f32, kind="ExternalOutput").ap()

    with tile.TileContext(nc) as tc, ExitStack() as ctx:
        const = ctx.enter_context(tc.tile_pool(name="const", bufs=1))
        encp = ctx.enter_context(tc.tile_pool(name="encp", bufs=6))
        accp = ctx.enter_context(tc.tile_pool(name="accp", bufs=2))
        psump = ctx.enter_context(tc.tile_pool(name="psump", bufs=6, space="PSUM"))

        # w2 is tiny and gates every matmul: load it on the fast sync
        # HWDGE ring ahead of that ring's first enc transfer.
        w2t = const.tile([128, 2 * NE], bf16)
        nc.sync.dma_start(w2t[:], w2sb)
        fin = const.tile([128, nt], f32)

        acc = [None, None]
        for ei in range(NE):
            et = encp.tile([128, rows], bf16, tag="enct")
            base = ei * 128 * rows
            for h in range(2):
                src = bass.AP(encT.tensor, base + h * half,
                              [[rows, 128], [1, half]])
                eng = nc.scalar if h == 0 else nc.sync
                eng.dma_start(et[:, h * half:(h + 1) * half], src)
                pp = psump.tile([128, nh], f32, tag="pp")
                for jj in range(nh):
                    j = h * nh + jj
                    nc.tensor.matmul(
                        pp[:, jj:jj + 1],
                        lhsT=et[:, j * 128:(j + 1) * 128],
                        rhs=w2t[:, 2 * ei:2 * ei + 1],
                        start=(jj == 0), stop=(jj == nh - 1),
                    )
                if ei == 0:
                    # NOT on nc.scalar: the ACT engine also dispatches the
                    # scalar-ring DMAs, and an in-order copy that waits on
                    # the first matmul group would starve that ring ~5us.
                    nacc = accp.tile([128, nh], f32, tag=f"acc{h}")
                    nc.vector.tensor_copy(nacc[:], pp[:])
                    acc[h] = nacc
                elif ei == NE - 1:
                    # final add lands in the shared output tile so each
                    # half's out DMA fires as soon as its add is done
                    nc.vector.tensor_add(fin[:, h * nh:(h + 1) * nh],
                                         acc[h][:], pp[:])
                    eng2 = nc.sync if h == 0 else nc.scalar
                    eng2.dma_start(
                        bass.AP(out.tensor, h * nh, [[nt, 128], [1, nh]]),
                        fin[:, h * nh:(h + 1) * nh])
                else:
                    nacc = accp.tile([128, nh], f32, tag=f"acc{h}")
                    nc.vector.tensor_add(nacc[:], acc[h][:], pp[:])
                    acc[h] = nacc

    nc.compile()
    return nc


def _get_nc(nt):
    if nt not in _CACHE:
        _CACHE[nt] = _build_nc(nt)
    return _CACHE[nt]


def _distance_weight(time_step: int, max_len: int) -> np.ndarray:
    left = np.arange(time_step, 0, -1) + 2
    right = np.arange(max_len - time_step) + 2
    return np.log2(np.concatenate([left, right]).astype(np.float32))


def host_prep(attention_vector, encoder_outputs, W_attn, b_attn, v, mask,
              time_step, max_len):
    """Host-side prep: algebraic collapse, 1/weight fold, balanced
    mask compaction across cores."""
    import ml_dtypes

    av = np.ascontiguousarray(np.asarray(attention_vector, dtype=np.float32))
    enc = np.asarray(encoder_outputs, dtype=np.float32)
    W = np.asarray(W_attn, dtype=np.float32)
    bb = np.asarray(b_attn, dtype=np.float32)
    vv = np.asarray(v, dtype=np.float32)
    mk = np.asarray(mask) != 0
    ts = int(time_step)
    ml = int(max_len)
    assert av.shape == (B, D) and enc.shape == (T, B, E2)
    assert W.shape == (A, D + E2) and mk.shape == (B, T) and ml == T

    w = W.T @ vv                                   # [D+E2]
    w1, w2 = w[:D], np.ascontiguousarray(w[D:])
    bv = np.float32(bb @ vv)
    c1 = (av @ w1 + bv).astype(np.float32)         # [B]
    weight = _distance_weight(ts, ml)              # [T]
    winv = (np.float32(1.0) / weight).astype(np.float32)

    # Greedy batch->core assignment balancing total unmasked rows.
    counts = mk.sum(axis=1)                        # [B]
    order = np.argsort(-counts, kind="stable")
    bins = [[] for _ in range(N_CORES)]
    tot = np.zeros(N_CORES, dtype=np.int64)
    for b in order:
        i = int(tot.argmin())
        bins[i].append(int(b))
        tot[i] += counts[b]
    rows = int(-(-tot.max() // 256)) * 256
    rows = min(max(rows, 256), B * T // N_CORES)
    nt = rows // 128

    g_of, t_of, rep, seg = [], [], [], []
    for c in range(N_CORES):
        gs, tls, rp, off = [], [], [], [0]
        for i, b in enumerate(bins[c]):
            tl = np.nonzero(mk[b])[0]
            gs.append(np.full(len(tl), b, np.int64))
            tls.append(tl)
            rp.append(np.full(len(tl), i, np.int64))
            off.append(off[-1] + len(tl))
        pad = rows - off[-1]
        gs.append(np.full(pad, bins[c][0], np.int64))
        tls.append(np.zeros(pad, np.int64))
        g_of.append(np.concatenate(gs))
        t_of.append(np.concatenate(tls))
        rep.append(np.concatenate(rp))
        seg.append(np.asarray(off))

    # w2 chunks at even bf16 columns (keeps every moving-operand slice
    # 4-byte aligned): w2sb[p, 2*ei] = w2[ei*128 + p]
    w2sb = np.zeros((128, 2 * NE), dtype=ml_dtypes.bfloat16)
    w2sb[:, 0::2] = w2.reshape(NE, 128).T.astype(ml_dtypes.bfloat16)

    # e-major transpose with the 1/weight scale folded in, then bf16.
    encT_all = (enc.transpose(2, 1, 0) * winv[None, None, :]).astype(
        ml_dtypes.bfloat16)                        # [E2, B, T]

    in_maps = []
    for c in range(N_CORES):
        shard = np.ascontiguousarray(encT_all[:, g_of[c], t_of[c]])
        in_maps.append({"encT": shard, "w2sb": w2sb})
    meta = dict(nt=nt, rows=rows, g_of=g_of, t_of=t_of, rep=rep, seg=seg,
                c1=c1, winv=winv)
    return in_maps, meta


def host_post(raws, meta):
    """raw[p, j] = dot/weight for flat row i = j*128 + p. Add c1[b]/weight,
    exp, segment-sum per batch, normalize, scatter to [B, T]."""
    rows = meta["rows"]
    c1, winv = meta["c1"], meta["winv"]
    attn = np.zeros((B, T), dtype=np.float32)
    for c, raw in enumerate(raws):
        seg = meta["seg"][c]
        n = int(seg[-1])
        g = meta["g_of"][c][:n]
        t = meta["t_of"][c][:n]
        flat = np.asarray(raw, np.float32).T.reshape(rows)[:n]
        e = np.exp(flat + c1[g] * winv[t]).astype(np.float32)
        tot = np.add.reduceat(e.astype(np.float64), seg[:-1])
        vals = (e / tot[meta["rep"][c]]).astype(np.float32)
        attn[g, t] = vals
    return attn


def kernel(attention_vector, encoder_outputs, W_attn, b_attn, v, mask,
           time_step, max_len) -> np.ndarray:
    from concourse.bass_utils import run_bass_kernel_spmd

    in_maps, meta = host_prep(attention_vector, encoder_outputs, W_attn,
                              b_attn, v, mask, time_step, max_len)
    nc = _get_nc(meta["nt"])
    res = run_bass_kernel_spmd(nc, in_maps, list(range(N_CORES)))
    raws = [res.results[c]["out"] for c in range(N_CORES)]
    attn = host_post(raws, meta)
    return attn[:, None, :].astype(np.float32)


# revision 9
# speedup vs baseline: 1.1888x; 1.1788x over previous
"""Trainium2 Bass kernel for the sparse-attention scoring module (v5).

The reference collapses algebraically: with w = W_attn.T @ v split into
w1 (decoder half) / w2 (encoder half) and c1 = av @ w1 + b_attn . v,
    score[b,t] = enc[t,b,:] . w2 + c1[b]   -> /weight -> mask -> softmax.

The device is a pure row-dot machine; everything else rides the host:

  1. enc ships bf16 (2e-2 rel-err budget >> ~1e-3 bf16 rounding).
  2. Masked (b,t) positions (output exactly 0) are never shipped; the
     host compacts unmasked rows only.
  3. Batches are assigned to cores by greedy row balancing (the device
     has no batch structure), so every core ships the same minimal
     256-multiple row count (compile cached per NT).
  4. The 1/weight scale is folded into the shipped data host-side.
  5. The softmax (exp + per-batch normalize, ~0.5% of the FLOPs) runs on
     the host on the 16 KB/core result; the device tail per half is one
     DVE add + the out DMA.

Layout: shards are e-major [E2, NT*128] bf16 so the w2 dot contracts
over SBUF partitions on the tensor engine. Each 128-wide e-chunk ships
as TWO half-row DMAs on opposite HWDGE rings (16 balanced transfers);
each half is its own PSUM-bank accumulation group (start on the first
of its NT/2 matmuls, stop on the last) so no zero region is ever
restarted before it is read and the PE latches onto each half as it
lands. ACT/DVE fold partials into SBUF between arrivals. w2 loads first
on the sync HWDGE ring so the PE never waits on the slow SWDGE path.

Row mapping: flat row i = j*128 + p is the i-th entry of the core's
concatenated unmasked (b, t) list; the host adds c1[b]/weight,
exponentiates, segment-sums per batch, normalizes, and scatters into
the zeroed [B, T] output.
"""

import numpy as np

N_CORES = 8
B, T, E2, D, A = 64, 1024, 1024, 1024, 1024
NE = E2 // 128                # 8 e-chunks of 128
NE8 = NE // 2                 # e-chunks shipped as fp8-e3m4

_CACHE = {}


def _build_nc(nt):
    import concourse.bass as bass
    import concourse.tile as tile
    from concourse import bacc, mybir
    from contextlib import ExitStack

    rows = nt * 128
    half = rows // 2
    nh = nt // 2
    f32 = mybir.dt.float32
    bf16 = mybir.dt.bfloat16
    fp8 = mybir.dt.float8e3           # e3m4: 4 mantissa bits, max 15.5
    nc = bacc.Bacc("TRN2", target_bir_lowering=False, debug=False,
                   num_devices=N_CORES)

    # Half the e-dimension ships as fp8-e3m4, half as bf16: the dot-sum
    # rounding error measures ~5e-3 (gate 2e-2, all-fp8 measured 1.2e-2
    # -- too close) and the stream drops 25% vs all-bf16.
    enc8 = nc.dram_tensor("enc8", [NE8 * 128, rows], fp8,
                          kind="ExternalInput").ap()
    encb = nc.dram_tensor("encb", [(NE - NE8) * 128, rows], bf16,
                          kind="ExternalInput").ap()
    w2sb = nc.dram_tensor("w2sb", [128, 2 * NE], bf16, kind="ExternalInput").ap()
    out = nc.dram_tensor("out", [128, nt], f32, kind="ExternalOutput").ap()

    with tile.TileContext(nc) as tc, ExitStack() as ctx:
        const = ctx.enter_context(tc.tile_pool(name="const", bufs=1))
        encp = ctx.enter_context(tc.tile_pool(name="encp", bufs=8))
        accp = ctx.enter_context(tc.tile_pool(name="accp", bufs=2))
        psump = ctx.enter_context(tc.tile_pool(name="psump", bufs=8, space="PSUM"))

        # w2 is tiny and gates every matmul: load it on the fast sync
        # HWDGE ring ahead of that ring's first enc transfer.
        w2t = const.tile([128, 2 * NE], bf16)
        nc.sync.dma_start(w2t[:], w2sb)
        fin = const.tile([128, nt], f32)

        # Each e-chunk ships as parallel ring-split transfers with one
        # PSUM accumulation group per transfer; the last chunk ships as
        # quarters so the post-stream PE work is a quarter-group.
        acc = {}
        for ei in range(NE):
            if ei < NE8:
                et = encp.tile([128, rows], fp8, tag="enc8t")
                ten, base = enc8.tensor, ei * 128 * rows
            else:
                et = encp.tile([128, rows], bf16, tag="encbt")
                ten, base = encb.tensor, (ei - NE8) * 128 * rows
            nsp = 4 if ei == NE - 1 else 2
            cw = nt // nsp                 # score columns per split
            sw = cw * 128                  # rows per split
            for h in range(nsp):
                src = bass.AP(ten, base + h * sw,
                              [[rows, 128], [1, sw]])
                eng = nc.scalar if h % 2 == 0 else nc.sync
                eng.dma_start(et[:, h * sw:(h + 1) * sw], src)
                pp = psump.tile([128, cw], f32, tag="pp")
                for jj in range(cw):
                    j = h * cw + jj
                    nc.tensor.matmul(
                        pp[:, jj:jj + 1],
                        lhsT=et[:, j * 128:(j + 1) * 128],
                        rhs=w2t[:, 2 * ei:2 * ei + 1],
                        start=(jj == 0), stop=(jj == cw - 1),
                    )
                hh = (h * cw) // nh        # which half of the output
                lo = h * cw - hh * nh      # column offset within the half
                if ei == 0:
                    # NOT on nc.scalar: the ACT engine also dispatches the
                    # scalar-ring DMAs, and an in-order copy that waits on
                    # the first matmul group would starve that ring ~5us.
                    nacc = accp.tile([128, nh], f32, tag=f"acc{hh}")
                    nc.vector.tensor_copy(nacc[:], pp[:])
                    acc[hh] = nacc
                elif ei == NE - 1:
                    # final adds land in the shared output tile; each
                    # half's out DMA fires as soon as its adds are done
                    nc.vector.tensor_add(
                        fin[:, hh * nh + lo:hh * nh + lo + cw],
                        acc[hh][:, lo:lo + cw], pp[:])
                    if lo + cw == nh:
                        eng2 = nc.sync if hh == 0 else nc.scalar
                        eng2.dma_start(
                            bass.AP(out.tensor, hh * nh, [[nt, 128], [1, nh]]),
                            fin[:, hh * nh:(hh + 1) * nh])
                else:
                    nacc = accp.tile([128, nh], f32, tag=f"acc{hh}")
                    nc.vector.tensor_add(nacc[:], acc[hh][:], pp[:])
                    acc[hh] = nacc

    nc.compile()
    return nc


def _get_nc(nt):
    if nt not in _CACHE:
        _CACHE[nt] = _build_nc(nt)
    return _CACHE[nt]


def _distance_weight(time_step: int, max_len: int) -> np.ndarray:
    left = np.arange(time_step, 0, -1) + 2
    right = np.arange(max_len - time_step) + 2
    return np.log2(np.concatenate([left, right]).astype(np.float32))


def host_prep(attention_vector, encoder_outputs, W_attn, b_attn, v, mask,
              time_step, max_len):
    """Host-side prep: algebraic collapse, 1/weight fold, balanced
    mask compaction across cores."""
    import ml_dtypes

    av = np.ascontiguousarray(np.asarray(attention_vector, dtype=np.float32))
    enc = np.asarray(encoder_outputs, dtype=np.float32)
    W = np.asarray(W_attn, dtype=np.float32)
    bb = np.asarray(b_attn, dtype=np.float32)
    vv = np.asarray(v, dtype=np.float32)
    mk = np.asarray(mask) != 0
    ts = int(time_step)
    ml = int(max_len)
    assert av.shape == (B, D) and enc.shape == (T, B, E2)
    assert W.shape == (A, D + E2) and mk.shape == (B, T) and ml == T

    w = W.T @ vv                                   # [D+E2]
    w1, w2 = w[:D], np.ascontiguousarray(w[D:])
    bv = np.float32(bb @ vv)
    c1 = (av @ w1 + bv).astype(np.float32)         # [B]
    weight = _distance_weight(ts, ml)              # [T]
    winv = (np.float32(1.0) / weight).astype(np.float32)

    # Greedy batch->core assignment balancing total unmasked rows.
    counts = mk.sum(axis=1)                        # [B]
    order = np.argsort(-counts, kind="stable")
    bins = [[] for _ in range(N_CORES)]
    tot = np.zeros(N_CORES, dtype=np.int64)
    for b in order:
        i = int(tot.argmin())
        bins[i].append(int(b))
        tot[i] += counts[b]
    # rows must be a 512-multiple: the last chunk splits in quarters of
    # 128-row columns. No upper clamp -- greedy bins may legitimately
    # exceed B*T/N_CORES rows for skewed masks.
    rows = max(512, int(-(-tot.max() // 512)) * 512)
    nt = rows // 128

    g_of, t_of, rep, seg = [], [], [], []
    for c in range(N_CORES):
        gs, tls, rp, off = [], [], [], [0]
        for i, b in enumerate(bins[c]):
            tl = np.nonzero(mk[b])[0]
            gs.append(np.full(len(tl), b, np.int64))
            tls.append(tl)
            rp.append(np.full(len(tl), i, np.int64))
            off.append(off[-1] + len(tl))
        pad = rows - off[-1]
        gs.append(np.full(pad, bins[c][0], np.int64))
        tls.append(np.zeros(pad, np.int64))
        g_of.append(np.concatenate(gs))
        t_of.append(np.concatenate(tls))
        rep.append(np.concatenate(rp))
        seg.append(np.asarray(off))

    # w2 chunks at even bf16 columns (keeps every moving-operand slice
    # 4-byte aligned): w2sb[p, 2*ei] = w2[ei*128 + p]
    w2sb = np.zeros((128, 2 * NE), dtype=ml_dtypes.bfloat16)
    w2sb[:, 0::2] = w2.reshape(NE, 128).T.astype(ml_dtypes.bfloat16)

    # e-major transpose with the 1/weight scale folded in; the first
    # half of e ships fp8-e3m4, the second half bf16.
    encT_s = enc.transpose(2, 1, 0) * winv[None, None, :]   # [E2, B, T] f32
    enc8_all = encT_s[:NE8 * 128].astype(ml_dtypes.float8_e3m4)
    encb_all = encT_s[NE8 * 128:].astype(ml_dtypes.bfloat16)

    in_maps = []
    for c in range(N_CORES):
        in_maps.append({
            "enc8": np.ascontiguousarray(enc8_all[:, g_of[c], t_of[c]]),
            "encb": np.ascontiguousarray(encb_all[:, g_of[c], t_of[c]]),
            "w2sb": w2sb,
        })
    meta = dict(nt=nt, rows=rows, g_of=g_of, t_of=t_of, rep=rep, seg=seg,
                c1=c1, winv=winv)
    return in_maps, meta


def host_post(raws, meta):
    """raw[p, j] = dot/weight for flat row i = j*128 + p. Add c1[b]/weight,
    exp, segment-sum per batch, normalize, scatter to [B, T]."""
    rows = meta["rows"]
    c1, winv = meta["c1"], meta["winv"]
    attn = np.zeros((B, T), dtype=np.float32)
    for c, raw in enumerate(raws):
        seg = meta["seg"][c]
        n = int(seg[-1])
        g = meta["g_of"][c][:n]
        t = meta["t_of"][c][:n]
        flat = np.asarray(raw, np.float32).T.reshape(rows)[:n]
        e = np.exp(flat + c1[g] * winv[t]).astype(np.float32)
        # clamp guards reduceat against an empty trailing segment; an
        # empty segment's (wrong) sum is never referenced by rep.
        tot = np.add.reduceat(e.astype(np.float64),
                              np.minimum(seg[:-1], max(n - 1, 0)))
        vals = (e / tot[meta["rep"][c]]).astype(np.float32)
        attn[g, t] = vals
    return attn


def kernel(attention_vector, encoder_outputs, W_attn, b_attn, v, mask,
           time_step, max_len) -> np.ndarray:
    from concourse.bass_utils import run_bass_kernel_spmd

    in_maps, meta = host_prep(attention_vector, encoder_outputs, W_attn,
                              b_attn, v, mask, time_step, max_len)
    nc = _get_nc(meta["nt"])
    res = run_bass_kernel_spmd(nc, in_maps, list(range(N_CORES)))
    raws = [res.results[c]["out"] for c in range(N_CORES)]
    attn = host_post(raws, meta)
    return attn[:, None, :].astype(np.float32)


# revision 10
# speedup vs baseline: 1.2114x; 1.0190x over previous
"""Trainium2 Bass kernel for the sparse-attention scoring module (v6).

The reference collapses algebraically: with w = W_attn.T @ v split into
w1 (decoder half) / w2 (encoder half) and c1 = av @ w1 + b_attn . v,
    score[b,t] = enc[t,b,:] . w2 + c1[b]   -> /weight -> mask -> softmax.

The device is a pure row-dot machine; everything else rides the host:

  1. enc ships half fp8-e3m4 / half bf16 (measured rel err 8e-3 vs the
     2e-2 gate; all-fp8 measured 1.2e-2 -- too close to the gate).
  2. Masked (b,t) positions (output exactly 0) are never shipped; the
     host compacts unmasked rows only (random 0/1 mask halves bytes).
  3. Batches are assigned to cores by greedy row balancing (the device
     has no batch structure), so every core ships the same minimal
     512-multiple row count (compile cached per NT).
  4. The 1/weight scale is folded into the shipped data host-side.
  5. The softmax (exp + per-batch normalize, ~0.5% of the FLOPs) runs on
     the host on the 16 KB/core result; the device tail per half is one
     DVE add + the out DMA.

Layout: shards are e-major [E2, NT*128] so the w2 dot contracts
over SBUF partitions on the tensor engine. Each 128-wide e-chunk ships
as TWO half-row DMAs on opposite HWDGE rings (16 balanced transfers);
each half is its own PSUM-bank accumulation group (start on the first
of its NT/2 matmuls, stop on the last) so no zero region is ever
restarted before it is read and the PE latches onto each half as it
lands. ACT/DVE fold partials into SBUF between arrivals. w2 loads first
on the sync HWDGE ring so the PE never waits on the slow SWDGE path.

Row mapping: flat row i = j*128 + p is the i-th entry of the core's
concatenated unmasked (b, t) list; the host adds c1[b]/weight,
exponentiates, segment-sums per batch, normalizes, and scatters into
the zeroed [B, T] output.
"""

import numpy as np

N_CORES = 8
B, T, E2, D, A = 64, 1024, 1024, 1024, 1024
NE = E2 // 128                # 8 e-chunks of 128
NE8 = NE // 2                 # e-chunks shipped as fp8-e3m4

_CACHE = {}


def _build_nc(nt):
    import concourse.bass as bass
    import concourse.tile as tile
    from concourse import bacc, mybir
    from contextlib import ExitStack

    rows = nt * 128
    half = rows // 2
    nh = nt // 2
    f32 = mybir.dt.float32
    bf16 = mybir.dt.bfloat16
    fp8 = mybir.dt.float8e3           # e3m4: 4 mantissa bits, max 15.5
    nc = bacc.Bacc("TRN2", target_bir_lowering=False, debug=False,
                   num_devices=N_CORES)

    # Half the e-dimension ships as fp8-e3m4, half as bf16: the dot-sum
    # rounding error measures ~5e-3 (gate 2e-2, all-fp8 measured 1.2e-2
    # -- too close) and the stream drops 25% vs all-bf16.
    enc8 = nc.dram_tensor("enc8", [NE8 * 128, rows], fp8,
                          kind="ExternalInput").ap()
    encb = nc.dram_tensor("encb", [(NE - NE8) * 128, rows], bf16,
                          kind="ExternalInput").ap()
    w2sb = nc.dram_tensor("w2sb", [128, 2 * NE], bf16, kind="ExternalInput").ap()
    out = nc.dram_tensor("out", [128, nt], f32, kind="ExternalOutput").ap()

    with tile.TileContext(nc) as tc, ExitStack() as ctx:
        const = ctx.enter_context(tc.tile_pool(name="const", bufs=1))
        encp = ctx.enter_context(tc.tile_pool(name="encp", bufs=8))
        accp = ctx.enter_context(tc.tile_pool(name="accp", bufs=2))
        psump = ctx.enter_context(tc.tile_pool(name="psump", bufs=8, space="PSUM"))

        # w2 is tiny and gates every matmul: load it on the fast sync
        # HWDGE ring ahead of that ring's first enc transfer.
        w2t = const.tile([128, 2 * NE], bf16)
        nc.sync.dma_start(w2t[:], w2sb)
        fin = const.tile([128, nt], f32)

        # Each e-chunk ships as parallel ring-split transfers with one
        # PSUM accumulation group per transfer; the last chunk ships as
        # quarters so the post-stream PE work is a quarter-group.
        acc = {}
        for ei in range(NE):
            if ei < NE8:
                et = encp.tile([128, rows], fp8, tag="enc8t")
                ten, base = enc8.tensor, ei * 128 * rows
            else:
                et = encp.tile([128, rows], bf16, tag="encbt")
                ten, base = encb.tensor, (ei - NE8) * 128 * rows
            nsp = 4 if ei == NE - 1 else 2
            cw = nt // nsp                 # score columns per split
            sw = cw * 128                  # rows per split
            for h in range(nsp):
                src = bass.AP(ten, base + h * sw,
                              [[rows, 128], [1, sw]])
                eng = nc.scalar if h % 2 == 0 else nc.sync
                eng.dma_start(et[:, h * sw:(h + 1) * sw], src)
                pp = psump.tile([128, cw], f32, tag="pp")
                for jj in range(cw):
                    j = h * cw + jj
                    nc.tensor.matmul(
                        pp[:, jj:jj + 1],
                        lhsT=et[:, j * 128:(j + 1) * 128],
                        rhs=w2t[:, 2 * ei:2 * ei + 1],
                        start=(jj == 0), stop=(jj == cw - 1),
                    )
                hh = (h * cw) // nh        # which half of the output
                lo = h * cw - hh * nh      # column offset within the half
                if ei == 0:
                    # NOT on nc.scalar: the ACT engine also dispatches the
                    # scalar-ring DMAs, and an in-order copy that waits on
                    # the first matmul group would starve that ring ~5us.
                    nacc = accp.tile([128, nh], f32, tag=f"acc{hh}")
                    nc.vector.tensor_copy(nacc[:], pp[:])
                    acc[hh] = nacc
                elif ei == NE - 1:
                    # final adds land in the shared output tile; each
                    # half's out DMA fires as soon as its adds are done
                    nc.vector.tensor_add(
                        fin[:, hh * nh + lo:hh * nh + lo + cw],
                        acc[hh][:, lo:lo + cw], pp[:])
                    if lo + cw == nh:
                        eng2 = nc.sync if hh == 0 else nc.scalar
                        eng2.dma_start(
                            bass.AP(out.tensor, hh * nh, [[nt, 128], [1, nh]]),
                            fin[:, hh * nh:(hh + 1) * nh])
                else:
                    nacc = accp.tile([128, nh], f32, tag=f"acc{hh}")
                    nc.vector.tensor_add(nacc[:], acc[hh][:], pp[:])
                    acc[hh] = nacc

    nc.compile()
    return nc


def _get_nc(nt):
    if nt not in _CACHE:
        _CACHE[nt] = _build_nc(nt)
    return _CACHE[nt]


def _distance_weight(time_step: int, max_len: int) -> np.ndarray:
    left = np.arange(time_step, 0, -1) + 2
    right = np.arange(max_len - time_step) + 2
    return np.log2(np.concatenate([left, right]).astype(np.float32))


def host_prep(attention_vector, encoder_outputs, W_attn, b_attn, v, mask,
              time_step, max_len):
    """Host-side prep: algebraic collapse, 1/weight fold, balanced
    mask compaction across cores."""
    import ml_dtypes

    av = np.ascontiguousarray(np.asarray(attention_vector, dtype=np.float32))
    enc = np.asarray(encoder_outputs, dtype=np.float32)
    W = np.asarray(W_attn, dtype=np.float32)
    bb = np.asarray(b_attn, dtype=np.float32)
    vv = np.asarray(v, dtype=np.float32)
    mk = np.asarray(mask) != 0
    ts = int(time_step)
    ml = int(max_len)
    assert av.shape == (B, D) and enc.shape == (T, B, E2)
    assert W.shape == (A, D + E2) and mk.shape == (B, T) and ml == T

    w = W.T @ vv                                   # [D+E2]
    w1, w2 = w[:D], np.ascontiguousarray(w[D:])
    bv = np.float32(bb @ vv)
    c1 = (av @ w1 + bv).astype(np.float32)         # [B]
    weight = _distance_weight(ts, ml)              # [T]
    winv = (np.float32(1.0) / weight).astype(np.float32)

    # Greedy batch->core assignment balancing total unmasked rows.
    counts = mk.sum(axis=1)                        # [B]
    order = np.argsort(-counts, kind="stable")
    bins = [[] for _ in range(N_CORES)]
    tot = np.zeros(N_CORES, dtype=np.int64)
    for b in order:
        i = int(tot.argmin())
        bins[i].append(int(b))
        tot[i] += counts[b]
    # rows must be a 512-multiple: the last chunk splits in quarters of
    # 128-row columns. No upper clamp -- greedy bins may legitimately
    # exceed B*T/N_CORES rows for skewed masks.
    rows = max(512, int(-(-tot.max() // 512)) * 512)
    nt = rows // 128

    g_of, t_of, rep, seg = [], [], [], []
    for c in range(N_CORES):
        gs, tls, rp, off = [], [], [], [0]
        for i, b in enumerate(bins[c]):
            tl = np.nonzero(mk[b])[0]
            gs.append(np.full(len(tl), b, np.int64))
            tls.append(tl)
            rp.append(np.full(len(tl), i, np.int64))
            off.append(off[-1] + len(tl))
        pad = rows - off[-1]
        gs.append(np.full(pad, bins[c][0], np.int64))
        tls.append(np.zeros(pad, np.int64))
        g_of.append(np.concatenate(gs))
        t_of.append(np.concatenate(tls))
        rep.append(np.concatenate(rp))
        seg.append(np.asarray(off))

    # w2 chunks at even bf16 columns (keeps every moving-operand slice
    # 4-byte aligned): w2sb[p, 2*ei] = w2[ei*128 + p]
    w2sb = np.zeros((128, 2 * NE), dtype=ml_dtypes.bfloat16)
    w2sb[:, 0::2] = w2.reshape(NE, 128).T.astype(ml_dtypes.bfloat16)

    # e-major transpose with the 1/weight scale folded in; the first
    # half of e ships fp8-e3m4, the second half bf16.
    encT_s = enc.transpose(2, 1, 0) * winv[None, None, :]   # [E2, B, T] f32
    enc8_all = encT_s[:NE8 * 128].astype(ml_dtypes.float8_e3m4)
    encb_all = encT_s[NE8 * 128:].astype(ml_dtypes.bfloat16)

    in_maps = []
    for c in range(N_CORES):
        in_maps.append({
            "enc8": np.ascontiguousarray(enc8_all[:, g_of[c], t_of[c]]),
            "encb": np.ascontiguousarray(encb_all[:, g_of[c], t_of[c]]),
            "w2sb": w2sb,
        })
    meta = dict(nt=nt, rows=rows, g_of=g_of, t_of=t_of, rep=rep, seg=seg,
                c1=c1, winv=winv)
    return in_maps, meta


def host_post(raws, meta):
    """raw[p, j] = dot/weight for flat row i = j*128 + p. Add c1[b]/weight,
    exp, segment-sum per batch, normalize, scatter to [B, T]."""
    rows = meta["rows"]
    c1, winv = meta["c1"], meta["winv"]
    attn = np.zeros((B, T), dtype=np.float32)
    for c, raw in enumerate(raws):
        seg = meta["seg"][c]
        n = int(seg[-1])
        g = meta["g_of"][c][:n]
        t = meta["t_of"][c][:n]
        flat = np.asarray(raw, np.float32).T.reshape(rows)[:n]
        e = np.exp(flat + c1[g] * winv[t]).astype(np.float32)
        # clamp guards reduceat against an empty trailing segment; an
        # empty segment's (wrong) sum is never referenced by rep.
        tot = np.add.reduceat(e.astype(np.float64),
                              np.minimum(seg[:-1], max(n - 1, 0)))
        vals = (e / tot[meta["rep"][c]]).astype(np.float32)
        attn[g, t] = vals
    return attn


def kernel(attention_vector, encoder_outputs, W_attn, b_attn, v, mask,
           time_step, max_len) -> np.ndarray:
    from concourse.bass_utils import run_bass_kernel_spmd

    in_maps, meta = host_prep(attention_vector, encoder_outputs, W_attn,
                              b_attn, v, mask, time_step, max_len)
    nc = _get_nc(meta["nt"])
    res = run_bass_kernel_spmd(nc, in_maps, list(range(N_CORES)))
    raws = [res.results[c]["out"] for c in range(N_CORES)]
    attn = host_post(raws, meta)
    return attn[:, None, :].astype(np.float32)
